# revision 24
# baseline (speedup 1.0000x reference)
"""Trainium2 kernel for the algo/task performance-scan problem.

The lax.scan's only cross-step dependency is the 64 scalars sig[:, lx[l]]
read each step.  That scalar chain (O(A*L + L^2)) runs on the host in
float64.  Given per-step coefficients c[a,l] = eff[a] + s[a,l]*boost[a],
the field is a banded matmul

    result[a, l, t] = sum_{j<=l} mem[a]^(l-j) * c[a,j] * row_j[t]

followed by sig = tanh(result / (2*diff))  (2*sigmoid(x)-1 = tanh(x/2)).

Device design (single f16 pass, tasks sorted by difficulty):
  * 1/(2*diff[t]) folds into R' = task_matrix[lx]/(2 diff), so PSUM holds
    x = result/(2 diff) directly.
  * All matmul operands are f16.  f16 moving runs at 2 cycles/row on HW
    (~27.5us/core) but keeps the PE densely busy -- important because the
    HAM clock gate halves the PE clock whenever it sees idle windows --
    and needs only one copy of R' (1.75MB) and G (1MB), minimizing the
    input-DMA flood that serializes ahead of the first matmul.
  * All output is int8 (1 B/elem):
      - sorted task blocks 0-5: ACT tanh (PSUM f32 -> SBUF f16), then
        DVE *126.5 -> int8.  Max err 8.2e-3 (gate 2e-2).
      - blocks 6-7 (high difficulty): x itself is quantized with a
        rigorous host-side per-task bound (int8 = x * 127/Bx_t); host
        computes tanh.  One DVE op, no ACT work.  Max err 8.0e-3.
  * R' is stored as 7 overlapping 128-row j-chunks so every l-tile is
    one K=128 matmul (K=64 partition-offset matmuls fault at runtime).
"""

import sys

sys.path.insert(0, "/opt/trn_rl_repo")

import numpy as np

A, T, L = 64, 1024, 512
NCORES = 8
ACORE = A // NCORES          # 8 algos per core
LT = 64                      # l-tile size
NLT = L // LT                # 8 l-tiles
NTB = T // 128               # 8 task blocks (sorted by difficulty)
NG = 2                       # output groups (4 l-tiles each)
NX = 6                       # t-blocks >= NX use int8-x output (no tanh)

_CACHE = {}


def _build_program():
    import concourse.tile as tile
    from concourse import bacc, mybir

    nc = bacc.Bacc("TRN2", target_bir_lowering=False, debug=False,
                   enable_asserts=False, num_devices=NCORES)
    f32 = mybir.dt.float32
    f16 = mybir.dt.float16
    i8 = mybir.dt.int8

    rp0_in = nc.dram_tensor("rp0", [3, 128, T], f16, kind="ExternalInput").ap()
    rp1_in = nc.dram_tensor("rp1", [4, 128, T], f16, kind="ExternalInput").ap()
    g0_in = nc.dram_tensor("g0", [4, 128, ACORE * LT], f16,
                           kind="ExternalInput").ap()
    g1_in = nc.dram_tensor("g1", [4, 128, ACORE * LT], f16,
                           kind="ExternalInput").ap()
    dsc_in = nc.dram_tensor("dsc", [128, NTB - NX], f32,
                            kind="ExternalInput").ap()
    out8 = nc.dram_tensor("out8", [NG * NTB, 128, ACORE * 256], i8,
                          kind="ExternalOutput").ap()

    with tile.TileContext(nc) as tc:
        with tc.tile_pool(name="consts", bufs=1) as consts, \
             tc.tile_pool(name="stage", bufs=4) as stage, \
             tc.tile_pool(name="stage8", bufs=4) as stage8, \
             tc.tile_pool(name="ps", bufs=2, space="PSUM") as psp:

            # Pre-load the tanh ACT table during the input-DMA lead-in.
            wsrc = consts.tile([128, 64], f16, tag="warm")
            wdst = consts.tile([128, 64], f16, tag="warmout")
            nc.gpsimd.memset(wsrc[:], 0.0)
            nc.scalar.activation(wdst[:], wsrc[:],
                                 mybir.ActivationFunctionType.Tanh,
                                 scale=1.0)

            def bulk(tag, src, n, width):
                t_ = consts.tile([128, n * width], f16, tag=tag)
                nc.sync.dma_start(
                    t_[:].rearrange("p (c w) -> p c w", c=n),
                    src.rearrange("c p w -> p c w"))
                return t_

            rp0 = bulk("rp0", rp0_in, 3, T)
            g0t = bulk("g0", g0_in, 4, ACORE * LT)
            dsc = consts.tile([128, NTB - NX], f32, tag="dsc")
            nc.sync.dma_start(dsc[:], dsc_in)
            rp1 = bulk("rp1", rp1_in, 4, T)
            g1t = bulk("g1", g1_in, 4, ACORE * LT)

            # R' chunk for each l-tile: rp0 = [A0@j0, B0@j64, A1@j128],
            # rp1 = [B1@j192, A2@j256, B2@j320, A3@j384]
            lt_chunk = [(0, 0), (0, 0), (0, 1), (0, 2),
                        (1, 0), (1, 1), (1, 2), (1, 3)]

            def rchunk(lt):      # [128, T] slice
                half, i = lt_chunk[lt]
                rt = rp0 if half == 0 else rp1
                return rt[:, i * T:(i + 1) * T]

            W = ACORE * LT

            def gslice(lt):      # [128, ACORE*LT]
                gt = g0t if lt < 4 else g1t
                return gt[:, (lt % 4) * W:(lt % 4 + 1) * W]

            for g in range(NG):
                for tb in range(NTB):
                    ps = psp.tile([128, 4 * W], f32, tag="ps")
                    for sub in range(4):
                        lt = g * 4 + sub
                        psl = ps[:, sub * W:(sub + 1) * W]
                        rt = rchunk(lt)
                        nc.tensor.matmul(
                            psl, lhsT=rt[:, tb * 128:(tb + 1) * 128],
                            rhs=gslice(lt), start=True, stop=True)
                    idx = g * 8 + tb
                    last = idx == NG * NTB - 1
                    # psum free layout: s*W + a*64 + ll
                    # sbuf free layout: a*256 + s*64 + ll (contiguous runs)
                    ps_r = ps[:].rearrange("p (s a l) -> p s a l", s=4,
                                           a=ACORE)
                    ob = stage8.tile([128, ACORE * 256], i8, tag="ob")
                    if tb < NX:
                        th = stage.tile([128, ACORE * 256], f16, tag="th")
                        nc.scalar.activation(
                            th[:].rearrange("p (a s l) -> p s a l",
                                            a=ACORE, s=4),
                            ps_r, mybir.ActivationFunctionType.Tanh,
                            scale=1.0)
                        nc.vector.tensor_scalar(
                            ob[:], th[:], 126.5, None, mybir.AluOpType.mult)
                        nc.sync.dma_start(out8[idx], ob[:])
                    else:
                        scol = dsc[:, tb - NX:tb - NX + 1]
                        ob_r = ob[:].rearrange("p (a s l) -> p s a l",
                                               a=ACORE, s=4)
                        # final tile: drain+store in halves so the last DMA
                        # overlaps the last tensor_scalar
                        halves = ((0, 2), (2, 4)) if last else ((0, 4),)
                        for h0, h1 in halves:
                            nc.vector.tensor_scalar(
                                ob_r[:, h0:h1], ps_r[:, h0:h1], scol, None,
                                mybir.AluOpType.mult)
                            nc.sync.dma_start(
                                out8[idx].rearrange(
                                    "p (a sl) -> p a sl", a=ACORE)
                                [:, :, h0 * 64:h1 * 64],
                                ob[:].rearrange("p (a sl) -> p a sl",
                                                a=ACORE)
                                [:, :, h0 * 64:h1 * 64])

    nc.compile()
    return nc


def _host_chain(lx, task_matrix, task_difficulty, alg_efficiency,
                alg_memory, alg_experience_boost):
    """Exact (f64) scalar feedback chain; returns per-core input maps."""
    lx = np.asarray(lx).astype(np.int64)
    TM = np.asarray(task_matrix, dtype=np.float64)
    diff = np.asarray(task_difficulty, dtype=np.float64)
    eff = np.asarray(alg_efficiency, dtype=np.float64)
    mem = np.asarray(alg_memory, dtype=np.float64)
    boost = np.asarray(alg_experience_boost, dtype=np.float64)

    R = TM[lx]                     # [L, T]
    TM2 = R[:, lx]                 # [L, L]
    dlx = diff[lx]                 # [L]

    resS = np.zeros((A, L))
    c = np.empty((A, L))
    for l in range(L):
        s_l = 2.0 / (1.0 + np.exp(-resS[:, l] / dlx[l])) - 1.0
        c[:, l] = eff + s_l * boost
        resS = resS * mem[:, None] + c[:, l][:, None] * TM2[l][None, :]

    order = np.argsort(diff, kind="stable")
    dsort = diff[order]
    Rp = R[:, order] / (2.0 * dsort[None, :])     # [L, T] sorted tasks

    # rigorous per-task bound on |x| for the int8-x blocks
    cmax = np.abs(c).max()
    memmax = mem.max()
    b = np.zeros(T)
    bmax = np.zeros(T)
    for l in range(L):
        b = memmax * b + cmax * np.abs(Rp[l])
        bmax = np.maximum(bmax, b)
    s_t = 127.0 / np.maximum(bmax, 1e-6)          # int8 = round(x * s_t)
    dsc = np.ascontiguousarray(
        s_t[NX * 128:].reshape(NTB - NX, 128).T).astype(np.float32)

    # G[a, lt, jj, ll] = mem^(l-j) * c[a, j], j = js(lt)+jj, l = 64*lt+ll
    pmat = mem[:, None] ** np.arange(192)[None, :]        # [A, 192]
    G = np.zeros((A, NLT, 128, LT))
    for lt in range(NLT):
        js = 0 if lt == 0 else 64 * (lt - 1)
        jw = np.arange(js, js + 128)
        lmj = (np.arange(LT)[None, :] + 64 * lt) - jw[:, None]   # [128, LT]
        valid = lmj >= 0
        G[:, lt] = np.where(valid[None],
                            pmat[:, np.maximum(lmj, 0)] * c[:, jw][:, :, None],
                            0.0)

    Rh = Rp.astype(np.float16)
    rp = {"rp0": np.ascontiguousarray(
              np.stack([Rh[s:s + 128] for s in (0, 64, 128)])),
          "rp1": np.ascontiguousarray(
              np.stack([Rh[s:s + 128] for s in (192, 256, 320, 384)])),
          "dsc": dsc}

    in_maps = []
    for core in range(NCORES):
        blk = G[core * ACORE:(core + 1) * ACORE]     # [ACORE, NLT, 128, LT]
        gp = blk.transpose(1, 2, 0, 3).reshape(
            NLT, 128, ACORE * LT).astype(np.float16)
        in_maps.append({
            **rp,
            "g0": np.ascontiguousarray(gp[:4]),
            "g1": np.ascontiguousarray(gp[4:]),
        })
    return in_maps, order, s_t


def kernel(lx, task_matrix, task_difficulty, alg_efficiency, alg_memory,
           alg_experience_boost):
    from concourse.bass_utils import run_bass_kernel_spmd

    in_maps, order, s_t = _host_chain(
        lx, task_matrix, task_difficulty, alg_efficiency, alg_memory,
        alg_experience_boost)

    if "nc" not in _CACHE:
        _CACHE["nc"] = _build_program()
    nc = _CACHE["nc"]

    res = run_bass_kernel_spmd(nc, in_maps, core_ids=list(range(NCORES)),
                               trace=False)
    srt = np.empty((A, T, L), dtype=np.float32)   # sorted-task sig field
    for cidx in range(NCORES):
        d8 = res.results[cidx]["out8"]            # [16, 128, 2048] int8
        for idx in range(NG * NTB):
            g, tb = idx // 8, idx % 8
            arr = d8[idx].astype(np.float32)      # [128t, (a,s,ll)]
            if tb < NX:
                sig = arr / 126.5
            else:
                sig = np.tanh(arr / s_t[tb * 128:(tb + 1) * 128][:, None])
            sig = sig.reshape(128, ACORE, 256).transpose(1, 0, 2)
            srt[cidx * ACORE:(cidx + 1) * ACORE,
                tb * 128:(tb + 1) * 128,
                g * 256:(g + 1) * 256] = sig
    out = np.empty((A, T, L + 1), dtype=np.float32)
    out[:, :, 0] = 0.0
    out[:, order, 1:] = srt
    return out


# revision 25
# speedup vs baseline: 1.0077x; 1.0077x over previous
"""Trainium2 kernel for the algo/task performance-scan problem.

The lax.scan's only cross-step dependency is the 64 scalars sig[:, lx[l]]
read each step.  That scalar chain (O(A*L + L^2)) runs on the host in
float64.  Given per-step coefficients c[a,l] = eff[a] + s[a,l]*boost[a],
the field is a banded matmul

    result[a, l, t] = sum_{j<=l} mem[a]^(l-j) * c[a,j] * row_j[t]

followed by sig = tanh(result / (2*diff))  (2*sigmoid(x)-1 = tanh(x/2)).

Device design (single f16 pass, tasks sorted by difficulty):
  * 1/(2*diff[t]) folds into R' = task_matrix[lx]/(2 diff), so PSUM holds
    x = result/(2 diff) directly.
  * All matmul operands are f16.  f16 moving runs at 2 cycles/row on HW
    (~27.5us/core) but keeps the PE densely busy -- important because the
    HAM clock gate halves the PE clock whenever it sees idle windows --
    and needs only one copy of R' (1.75MB) and G (1MB), minimizing the
    input-DMA flood that serializes ahead of the first matmul.
  * All output is int8 (1 B/elem):
      - sorted task blocks 0-5: ACT tanh (PSUM f32 -> SBUF f16), then
        DVE *126.5 -> int8.  Max err 8.2e-3 (gate 2e-2).
      - blocks 6-7 (high difficulty): x itself is quantized with a
        rigorous host-side per-task bound (int8 = x * 127/Bx_t); host
        computes tanh.  One DVE op, no ACT work.  Max err 8.0e-3.
  * R' is stored as 7 overlapping 128-row j-chunks so every l-tile is
    one K=128 matmul (K=64 partition-offset matmuls fault at runtime).
"""

import sys

sys.path.insert(0, "/opt/trn_rl_repo")

import numpy as np

A, T, L = 64, 1024, 512
NCORES = 8
ACORE = A // NCORES          # 8 algos per core
LT = 64                      # l-tile size
NLT = L // LT                # 8 l-tiles
NTB = T // 128               # 8 task blocks (sorted by difficulty)
NG = 2                       # output groups (4 l-tiles each)
NX = 6                       # t-blocks >= NX use int8-x output (no tanh)

_CACHE = {}


def _build_program():
    import concourse.tile as tile
    from concourse import bacc, mybir

    nc = bacc.Bacc("TRN2", target_bir_lowering=False, debug=False,
                   enable_asserts=False, num_devices=NCORES)
    f32 = mybir.dt.float32
    f16 = mybir.dt.float16
    i8 = mybir.dt.int8

    rp0_in = nc.dram_tensor("rp0", [3, 128, T], f16, kind="ExternalInput").ap()
    rp1_in = nc.dram_tensor("rp1", [4, 128, T], f16, kind="ExternalInput").ap()
    g0_in = nc.dram_tensor("g0", [4, 128, ACORE * LT], f16,
                           kind="ExternalInput").ap()
    g1_in = nc.dram_tensor("g1", [4, 128, ACORE * LT], f16,
                           kind="ExternalInput").ap()
    dsc_in = nc.dram_tensor("dsc", [128, NTB - NX], f32,
                            kind="ExternalInput").ap()
    out8 = nc.dram_tensor("out8", [NG * NTB, 128, ACORE * 256], i8,
                          kind="ExternalOutput").ap()

    with tile.TileContext(nc) as tc:
        with tc.tile_pool(name="consts", bufs=1) as consts, \
             tc.tile_pool(name="stage", bufs=4) as stage, \
             tc.tile_pool(name="stage8", bufs=4) as stage8, \
             tc.tile_pool(name="ps", bufs=2, space="PSUM") as psp:

            # Pre-load the tanh ACT table during the input-DMA lead-in.
            wsrc = consts.tile([128, 64], f16, tag="warm")
            wdst = consts.tile([128, 64], f16, tag="warmout")
            nc.gpsimd.memset(wsrc[:], 0.0)
            nc.scalar.activation(wdst[:], wsrc[:],
                                 mybir.ActivationFunctionType.Tanh,
                                 scale=1.0)

            def bulk(tag, src, n, width):
                t_ = consts.tile([128, n * width], f16, tag=tag)
                nc.sync.dma_start(
                    t_[:].rearrange("p (c w) -> p c w", c=n),
                    src.rearrange("c p w -> p c w"))
                return t_

            rp0 = bulk("rp0", rp0_in, 3, T)
            g0t = bulk("g0", g0_in, 4, ACORE * LT)
            dsc = consts.tile([128, NTB - NX], f32, tag="dsc")
            nc.sync.dma_start(dsc[:], dsc_in)
            rp1 = bulk("rp1", rp1_in, 4, T)
            g1t = bulk("g1", g1_in, 4, ACORE * LT)

            # R' chunk for each l-tile: rp0 = [A0@j0, B0@j64, A1@j128],
            # rp1 = [B1@j192, A2@j256, B2@j320, A3@j384]
            lt_chunk = [(0, 0), (0, 0), (0, 1), (0, 2),
                        (1, 0), (1, 1), (1, 2), (1, 3)]

            def rchunk(lt):      # [128, T] slice
                half, i = lt_chunk[lt]
                rt = rp0 if half == 0 else rp1
                return rt[:, i * T:(i + 1) * T]

            W = ACORE * LT

            def gslice(lt):      # [128, ACORE*LT]
                gt = g0t if lt < 4 else g1t
                return gt[:, (lt % 4) * W:(lt % 4 + 1) * W]

            for g in range(NG):
                for tb in range(NTB):
                    ps = psp.tile([128, 4 * W], f32, tag="ps")
                    for sub in range(4):
                        lt = g * 4 + sub
                        psl = ps[:, sub * W:(sub + 1) * W]
                        rt = rchunk(lt)
                        nc.tensor.matmul(
                            psl, lhsT=rt[:, tb * 128:(tb + 1) * 128],
                            rhs=gslice(lt), start=True, stop=True)
                    idx = g * 8 + tb
                    last = idx == NG * NTB - 1
                    # psum free layout: s*W + a*64 + ll
                    # sbuf free layout: a*256 + s*64 + ll (contiguous runs)
                    ps_r = ps[:].rearrange("p (s a l) -> p s a l", s=4,
                                           a=ACORE)
                    ob = stage8.tile([128, ACORE * 256], i8, tag="ob")
                    if tb < NX:
                        th = stage.tile([128, ACORE * 256], f16, tag="th")
                        nc.scalar.activation(
                            th[:].rearrange("p (a s l) -> p s a l",
                                            a=ACORE, s=4),
                            ps_r, mybir.ActivationFunctionType.Tanh,
                            scale=1.0)
                        nc.vector.tensor_scalar(
                            ob[:], th[:], 126.5, None, mybir.AluOpType.mult)
                        nc.sync.dma_start(out8[idx], ob[:])
                    else:
                        scol = dsc[:, tb - NX:tb - NX + 1]
                        ob_r = ob[:].rearrange("p (a s l) -> p s a l",
                                               a=ACORE, s=4)
                        nc.vector.tensor_scalar(
                            ob_r, ps_r, scol, None, mybir.AluOpType.mult)
                        nc.sync.dma_start(out8[idx], ob[:])

    nc.compile()
    return nc


def _host_chain(lx, task_matrix, task_difficulty, alg_efficiency,
                alg_memory, alg_experience_boost):
    """Exact (f64) scalar feedback chain; returns per-core input maps."""
    lx = np.asarray(lx).astype(np.int64)
    TM = np.asarray(task_matrix, dtype=np.float64)
    diff = np.asarray(task_difficulty, dtype=np.float64)
    eff = np.asarray(alg_efficiency, dtype=np.float64)
    mem = np.asarray(alg_memory, dtype=np.float64)
    boost = np.asarray(alg_experience_boost, dtype=np.float64)

    R = TM[lx]                     # [L, T]
    TM2 = R[:, lx]                 # [L, L]
    dlx = diff[lx]                 # [L]

    resS = np.zeros((A, L))
    c = np.empty((A, L))
    for l in range(L):
        s_l = 2.0 / (1.0 + np.exp(-resS[:, l] / dlx[l])) - 1.0
        c[:, l] = eff + s_l * boost
        resS = resS * mem[:, None] + c[:, l][:, None] * TM2[l][None, :]

    order = np.argsort(diff, kind="stable")
    dsort = diff[order]
    Rp = R[:, order] / (2.0 * dsort[None, :])     # [L, T] sorted tasks

    # rigorous per-task bound on |x| for the int8-x blocks
    cmax = np.abs(c).max()
    memmax = mem.max()
    b = np.zeros(T)
    bmax = np.zeros(T)
    for l in range(L):
        b = memmax * b + cmax * np.abs(Rp[l])
        bmax = np.maximum(bmax, b)
    s_t = 127.0 / np.maximum(bmax, 1e-6)          # int8 = round(x * s_t)
    dsc = np.ascontiguousarray(
        s_t[NX * 128:].reshape(NTB - NX, 128).T).astype(np.float32)

    # G[a, lt, jj, ll] = mem^(l-j) * c[a, j], j = js(lt)+jj, l = 64*lt+ll
    pmat = mem[:, None] ** np.arange(192)[None, :]        # [A, 192]
    G = np.zeros((A, NLT, 128, LT))
    for lt in range(NLT):
        js = 0 if lt == 0 else 64 * (lt - 1)
        jw = np.arange(js, js + 128)
        lmj = (np.arange(LT)[None, :] + 64 * lt) - jw[:, None]   # [128, LT]
        valid = lmj >= 0
        G[:, lt] = np.where(valid[None],
                            pmat[:, np.maximum(lmj, 0)] * c[:, jw][:, :, None],
                            0.0)

    Rh = Rp.astype(np.float16)
    rp = {"rp0": np.ascontiguousarray(
              np.stack([Rh[s:s + 128] for s in (0, 64, 128)])),
          "rp1": np.ascontiguousarray(
              np.stack([Rh[s:s + 128] for s in (192, 256, 320, 384)])),
          "dsc": dsc}

    in_maps = []
    for core in range(NCORES):
        blk = G[core * ACORE:(core + 1) * ACORE]     # [ACORE, NLT, 128, LT]
        gp = blk.transpose(1, 2, 0, 3).reshape(
            NLT, 128, ACORE * LT).astype(np.float16)
        in_maps.append({
            **rp,
            "g0": np.ascontiguousarray(gp[:4]),
            "g1": np.ascontiguousarray(gp[4:]),
        })
    return in_maps, order, s_t


def kernel(lx, task_matrix, task_difficulty, alg_efficiency, alg_memory,
           alg_experience_boost):
    from concourse.bass_utils import run_bass_kernel_spmd

    in_maps, order, s_t = _host_chain(
        lx, task_matrix, task_difficulty, alg_efficiency, alg_memory,
        alg_experience_boost)

    if "nc" not in _CACHE:
        _CACHE["nc"] = _build_program()
    nc = _CACHE["nc"]

    res = run_bass_kernel_spmd(nc, in_maps, core_ids=list(range(NCORES)),
                               trace=False)
    srt = np.empty((A, T, L), dtype=np.float32)   # sorted-task sig field
    for cidx in range(NCORES):
        d8 = res.results[cidx]["out8"]            # [16, 128, 2048] int8
        for idx in range(NG * NTB):
            g, tb = idx // 8, idx % 8
            arr = d8[idx].astype(np.float32)      # [128t, (a,s,ll)]
            if tb < NX:
                sig = arr / 126.5
            else:
                sig = np.tanh(arr / s_t[tb * 128:(tb + 1) * 128][:, None])
            sig = sig.reshape(128, ACORE, 256).transpose(1, 0, 2)
            srt[cidx * ACORE:(cidx + 1) * ACORE,
                tb * 128:(tb + 1) * 128,
                g * 256:(g + 1) * 256] = sig
    out = np.empty((A, T, L + 1), dtype=np.float32)
    out[:, :, 0] = 0.0
    out[:, order, 1:] = srt
    return out
